# revision 1
# baseline (speedup 1.0000x reference)
"""TRN2 Bass kernel for DenseDilatedKnnGraph (B=4, C=64, N=4096, k=9, dilation=2).

Algorithm
---------
reference: xt (B,N,C); dist(i,j) = |xi|^2 - 2<xi,xj> + |xj|^2; nn_idx = top-18
of -dist per row (stable, lowest-index tie-break); output nn_idx[..., ::2] plus
a center-index row -> (2, B, N, 9) int32.

Per-row ordering of -dist is identical to the ordering of
    s_ij = 2<xi,xj> - |xj|^2
(the |xi|^2 term is constant per row), and s has better relative precision.

Device (per core, SPMD over 8 cores; core = (batch, query-half)):
  - s computed via 2 fp16 K=128 matmuls (hi/lo split of fp32, error ~1e-6,
    ~4x cheaper than native fp32 matmul on the PE; K=128 keeps the PE at
    1 cycle/column — K<=64 matmuls stream at half rate):
      s = (qh@ch + ql@ch) + (qh@cl + s1+s2+s3)
    matmul A: stationary [qh; ql] (128 x 128), moving [ch; ch] (128 x 512)
    matmul B: stationary [qh; 1,1,1, 0...] , moving [cl; s1; s2; s3; junk]
    where qh/ql = fp16 split of 2x (queries), ch/cl = fp16 split of x
    (candidates), s1..s3 = 3-level fp16 split of -|xj|^2. The zero rows of
    B's stationary null out the junk rows of its moving operand. PSUM fp32
    accumulate, 128-query tiles, 512-wide PSUM chunks.
  - PSUM -> SBUF copy on the scalar engine.
  - DVE top-k: per GROUP-wide group max8 (values) + max_index (local indices,
    first-occurrence = lowest-index tie-break, matching jax.lax.top_k).
  - DMA out: group-candidate values U (128 x UW), local indices L (128 x UW).

Host: one stable argsort of each row's UW group-candidates (slot order ==
global index order for equal values, preserving the stable tie-break) yields
the top-18 global indices per row; this merge is 64->18 bookkeeping on
device-selected candidates (the 4096->UW selection ran on device). Rows where
a single group contributed 8 members to the top-18 (its 9th member could have
been lost) are recomputed exactly on the host (~1300 of 16384 rows).
"""

import numpy as np

import concourse.bacc as bacc
import concourse.mybir as mybir
import concourse.tile as tile
from concourse.bass_utils import run_bass_kernel_spmd

# Problem constants (hardcoded per harness contract).
B = 4
C = 64
N = 4096
K = 9
DILATION = 2
K_EFF = K * DILATION      # 18
P = 128                   # partitions / queries per tile
KM = 128                  # matmul contraction (keeps PE in full-rate mode)
# DVE max8 group boundaries. Fewer/wider groups cut per-instruction DVE
# overhead (the 2 full passes over S are fixed cost) but raise the rate of
# hazard rows (a group contributing 8 of the top-18 needs a host recompute):
# 8x512 -> 114 rows (185.6us), 6x~683 -> 526 (179.3us), 5x~820 -> 1281
# (175.3us), 4x1024 -> 3726 rows of 16384 (171.4us, rejected: 23% repairs).
GROUP_BOUNDS = (0, 820, 1640, 2460, 3280, 4096)
NG = len(GROUP_BOUNDS) - 1
UW = NG * 8               # group-candidates per row
N_CORES = 8
QROWS = (B * N) // N_CORES          # 2048 query rows per core
N_TILES = QROWS // P                # 16 tiles per core


def _build_program(n_tiles=N_TILES):
    nc = bacc.Bacc(
        "TRN2", target_bir_lowering=False, debug=False, enable_asserts=False
    )
    f32 = mybir.dt.float32
    f16 = mybir.dt.float16
    u32 = mybir.dt.uint32
    nq = n_tiles * P
    lhs_a = nc.dram_tensor("lhs_a", (KM, nq), f16, kind="ExternalInput")
    lhs_b = nc.dram_tensor("lhs_b", (KM, nq), f16, kind="ExternalInput")
    rhs_a = nc.dram_tensor("rhs_a", (KM, N), f16, kind="ExternalInput")
    rhs_b = nc.dram_tensor("rhs_b", (KM, N), f16, kind="ExternalInput")
    u_out = nc.dram_tensor("u_out", (nq, UW), f32, kind="ExternalOutput")
    l_out = nc.dram_tensor("l_out", (nq, UW), u32, kind="ExternalOutput")
    lhs_a_ap, lhs_b_ap = lhs_a.ap(), lhs_b.ap()
    rhs_a_ap, rhs_b_ap = rhs_a.ap(), rhs_b.ap()
    u_ap, l_ap = u_out.ap(), l_out.ap()

    with tile.TileContext(nc) as tc:
        with (
            tc.tile_pool(name="const", bufs=1) as cpool,
            tc.tile_pool(name="psum", bufs=2, space="PSUM") as ppool,
            tc.tile_pool(name="work", bufs=4) as wpool,
            tc.tile_pool(name="outp", bufs=4) as opool,
        ):
            # dependency-free warm-up matmuls that run during the input-DMA
            # prologue (nudges the PE toward its full-rate mode before the
            # real K=128 stream starts; otherwise free)
            prime = cpool.tile([KM, 512], f16)
            nc.gpsimd.memset(prime[:, :], 0.0)
            pps = ppool.tile([P, N // 2], f32, tag="ps")
            for _ in range(12):
                nc.tensor.matmul(pps[:, :512], prime[:, :128], prime[:, :],
                                 start=True, stop=True)

            # per-512-column-chunk input tiles: the first matmul only waits
            # for its own 128KB chunk, not the whole 2MB load
            ra_sb = [
                cpool.tile([KM, 512], f16, name=f"ra{j}", tag=f"ra{j}")
                for j in range(8)
            ]
            rb_sb = [
                cpool.tile([KM, 512], f16, name=f"rb{j}", tag=f"rb{j}")
                for j in range(8)
            ]
            la_sb = cpool.tile([KM, nq], f16)
            lb_sb = cpool.tile([KM, nq], f16)
            # tile 0 needs la/lb chunk 0 + ra0/rb0 first; issue those before
            # the rest so the first matmul isn't gated on the whole load
            w0 = min(512, nq)
            nc.sync.dma_start(la_sb[:, 0:w0], lhs_a_ap[:, 0:w0])
            nc.sync.dma_start(lb_sb[:, 0:w0], lhs_b_ap[:, 0:w0])
            nc.sync.dma_start(ra_sb[0][:, :], rhs_a_ap[:, 0:512])
            nc.sync.dma_start(rb_sb[0][:, :], rhs_b_ap[:, 0:512])
            for j in range(1, 8):
                nc.sync.dma_start(ra_sb[j][:, :], rhs_a_ap[:, j * 512 : (j + 1) * 512])
                nc.sync.dma_start(rb_sb[j][:, :], rhs_b_ap[:, j * 512 : (j + 1) * 512])
            for j in range(512, nq, 512):
                w = min(512, nq - j)
                nc.sync.dma_start(la_sb[:, j : j + w], lhs_a_ap[:, j : j + w])
                nc.sync.dma_start(lb_sb[:, j : j + w], lhs_b_ap[:, j : j + w])

            for t in range(n_tiles):
                ssb = wpool.tile([P, N], f32, tag="ssb")
                qs = slice(t * P, (t + 1) * P)
                for h in range(2):
                    ps = ppool.tile([P, N // 2], f32, tag="ps")
                    for j in range(4):
                        cj = h * 4 + j
                        pslice = ps[:, j * 512 : (j + 1) * 512]
                        nc.tensor.matmul(
                            pslice, la_sb[:, qs], ra_sb[cj][:, :],
                            start=True, stop=False,
                        )
                        nc.tensor.matmul(
                            pslice, lb_sb[:, qs], rb_sb[cj][:, :],
                            start=False, stop=True,
                        )
                    for cc in range(4):
                        nc.scalar.copy(
                            ssb[:, h * (N // 2) + cc * 512 : h * (N // 2) + (cc + 1) * 512],
                            ps[:, cc * 512 : (cc + 1) * 512],
                        )

                u = opool.tile([P, UW], f32, tag="u")
                l = opool.tile([P, UW], u32, tag="l")
                for g in range(NG):
                    nc.vector.max(
                        out=u[:, g * 8 : (g + 1) * 8],
                        in_=ssb[:, GROUP_BOUNDS[g] : GROUP_BOUNDS[g + 1]],
                    )
                for g in range(NG):
                    nc.vector.max_index(
                        out=l[:, g * 8 : (g + 1) * 8],
                        in_max=u[:, g * 8 : (g + 1) * 8],
                        in_values=ssb[:, GROUP_BOUNDS[g] : GROUP_BOUNDS[g + 1]],
                    )

                rs = slice(t * P, (t + 1) * P)
                nc.sync.dma_start(u_ap[rs, :], u[:])
                nc.sync.dma_start(l_ap[rs, :], l[:])
    nc.compile()
    return nc


def _split16(a):
    hi = a.astype(np.float16)
    lo = (a - hi.astype(np.float32)).astype(np.float16)
    return hi, lo


def _prep_core_inputs(X, core):
    """X: (B, N, C) fp32. Returns input map for one core."""
    b, h = divmod(core, N_CORES // B)
    Xb = X[b]
    xsq = np.sum(Xb * Xb, axis=1, dtype=np.float32)
    ch, cl = _split16(Xb.T)                       # (C, N) fp16 each
    # 3-level fp16 split of -xsq
    s1 = (-xsq).astype(np.float16)
    r = -xsq - s1.astype(np.float32)
    s2 = r.astype(np.float16)
    s3 = (r - s2.astype(np.float32)).astype(np.float16)
    # matmul A: s_partial = qh@ch + ql@ch ; moving = [ch; ch]
    rhs_a = np.empty((KM, N), np.float16)
    rhs_a[:C] = ch
    rhs_a[C:] = ch
    # matmul B: += qh@cl + s1+s2+s3 ; moving = [cl; s1; s2; s3; zeros]
    rhs_b = np.zeros((KM, N), np.float16)
    rhs_b[:C] = cl
    rhs_b[C] = s1
    rhs_b[C + 1] = s2
    rhs_b[C + 2] = s3

    Q = 2.0 * Xb[h * QROWS : (h + 1) * QROWS]     # (QROWS, C)
    qh, ql = _split16(Q.T)                        # (C, QROWS)
    lhs_a = np.empty((KM, QROWS), np.float16)
    lhs_a[:C] = qh
    lhs_a[C:] = ql
    lhs_b = np.zeros((KM, QROWS), np.float16)
    lhs_b[:C] = qh
    lhs_b[C : C + 3] = 1.0
    return {"lhs_a": lhs_a, "lhs_b": lhs_b, "rhs_a": rhs_a, "rhs_b": rhs_b}


def _merge_ranks(U, L):
    """Merge each row's UW device-selected candidates (values U, local idx L)
    into the top-18 global indices. Slot order within equal values == global
    index order, so a stable sort reproduces jax.lax.top_k tie-breaking.
    Returns (idx (R,18) int64, bad-row mask (R,))."""
    R = U.shape[0]
    g_of_slot = np.asarray(GROUP_BOUNDS[:-1], dtype=np.int64)[
        np.arange(UW) // 8
    ]
    Gidx = L.astype(np.int64) + g_of_slot[None, :]
    order = np.argsort(-U, axis=1, kind="stable")[:, :K_EFF]   # top-18 slots
    out = np.take_along_axis(Gidx, order, axis=1)
    # hazard: a group whose full top-8 landed in the top-18 may have lost a
    # 9th member that belongs there
    grp = order // 8
    counts = np.zeros((R, NG), np.int32)
    np.add.at(counts, (np.repeat(np.arange(R), K_EFF), grp.ravel()), 1)
    bad = (counts >= 8).any(axis=1)
    return out, bad


_NC_CACHE = {}


def kernel(x: np.ndarray) -> np.ndarray:
    x = np.asarray(x)
    assert x.shape == (B, C, N, 1), x.shape
    X = np.ascontiguousarray(np.transpose(x[..., 0], (0, 2, 1)))  # (B, N, C)

    if N_TILES not in _NC_CACHE:
        _NC_CACHE[N_TILES] = _build_program(N_TILES)
    nc = _NC_CACHE[N_TILES]

    in_maps = [_prep_core_inputs(X, c) for c in range(N_CORES)]
    res = run_bass_kernel_spmd(nc, in_maps, core_ids=list(range(N_CORES)))

    nn_idx = np.empty((B, N, K_EFF), np.int64)
    bad_rows = [[] for _ in range(B)]
    for core in range(N_CORES):
        b, h = divmod(core, N_CORES // B)
        r = res.results[core]
        idx, bad = _merge_ranks(r["u_out"], r["l_out"])
        nn_idx[b, h * QROWS : (h + 1) * QROWS] = idx
        if bad.any():
            bad_rows[b].extend((h * QROWS + np.nonzero(bad)[0]).tolist())

    # vectorized host repair of hazard rows (exact fp32 recompute)
    for b in range(B):
        if not bad_rows[b]:
            continue
        rows = np.asarray(sorted(bad_rows[b]))
        Xb = X[b]
        xsq = np.sum(Xb * Xb, axis=1, dtype=np.float32)
        S = (2.0 * Xb[rows]) @ Xb.T
        S = (S - xsq[None, :]).astype(np.float32)
        order = np.argsort(-S, axis=1, kind="stable")
        nn_idx[b, rows] = order[:, :K_EFF]

    nn_dil = nn_idx[:, :, ::DILATION]                       # (B, N, 9)
    center = np.broadcast_to(np.arange(N)[None, :, None], nn_dil.shape)
    out = np.stack((nn_dil, center), axis=0).astype(np.int32)
    return out



# revision 7
# speedup vs baseline: 1.4999x; 1.4999x over previous
"""TRN2 Bass kernel for DenseDilatedKnnGraph (B=4, C=64, N=4096, k=9, dilation=2).

Algorithm v2 (tournament-tree candidate selection + exact host rescore)
----------------------------------------------------------------------
reference: xt (B,N,C); dist(i,j) = |xi|^2 - 2<xi,xj> + |xj|^2; nn_idx = top-18
of -dist per row (stable, lowest-index tie-break); output nn_idx[..., ::2] plus
a center-index row -> (2, B, N, 9) int32.

Per-row ordering of -dist equals the ordering of s_ij = 2<xi,xj> - |xj|^2.
The device computes an APPROXIMATE s~ (single fp16 matmul, error ~0.01) that
is only used to SELECT candidate columns; the host rescores candidates in
fp64, so device values never need to be exact.

Device (per core, SPMD over 8 cores; core = (batch, query-half)):
  - s~ via ONE fp16 K=128 matmul into PSUM fp32:
      stationary [qh(64); 1; 1; 0...], moving [ch(64); s1; s2; junk]
    where qh = fp16(2x_q), ch = fp16(x_c), s1+s2 = 2-level fp16 split of
    -|x_c|^2 (junk rows nulled by zero stationary rows). 128-query tiles,
    512-wide PSUM chunks, [128,2048] PSUM buffers x2.
  - Tournament max tree (values preserved exactly through fp32 max):
      L1 (DVE):    T1[j] = max(ps[j], cs[j]) per half, where cs = scalar-engine
                   copy of ps[:, 1024:2048] (the ISA allows only ONE PSUM
                   operand per TensorTensor; tensor_tensor reads 2 ops/cycle:
                   2x cheaper than MAX8 scans)
      L2..L4 (DVE): T2[j] = max(T1[j], T1[j+1024]); T3[j] = max(T2[j],
                   T2[j+512]); T4[j] = max(T3[j], T3[j+256]) -> T4 256 wide,
                   T4[j] covers original columns {j + 256k, k=0..15}.
                   (TensorTensor does not lower on the Pool/GpSimd engine,
                   so the whole tree lives on DVE.)
  - DVE max8 (top-8 values per T4 group) + max_index (first-occurrence local
    slot) on the NARROW T4 only: NG groups over [0,256).
  - DMA out: local slot indices L (128 x NG*8 uint16). Values stay on device.

Host: slot -> T4 position p -> 16 candidate columns {p + 256k}. Rescore all
48*16 = 768 candidates per row exactly (fp64 BLAS), stable top-18 by
(value desc, col asc) == jax.lax.top_k ordering. Soundness: a true top-18
member's T4 slot is either selected (margin ~100 sigma) or its row is flagged
for full exact repair by
  (a) margin rule: some group's 8 slot-winners all within MARGIN=0.5 of the
      row's 18th-best (device noise < 0.12 worst case), or
  (b) duplicate-slot rule: find_index returned the same slot twice (fp32
      value tie collapsed a slot).
Flagged rows (~4-7%) get a full 4096-wide fp64 recompute on host.
"""

import numpy as np

import concourse.bacc as bacc
import concourse.mybir as mybir
import concourse.tile as tile
from concourse.bass_utils import run_bass_kernel_spmd

# Problem constants (hardcoded per harness contract).
B = 4
C = 64
N = 4096
K = 9
DILATION = 2
K_EFF = K * DILATION      # 18
P = 128                   # partitions / queries per tile
KM = 128                  # matmul contraction rows (K>=65 keeps PE full rate)
N_CORES = 8
QROWS = (B * N) // N_CORES          # 2048 query rows per core
N_TILES = QROWS // P                # 16 tiles per core

TOPW = 256                # tournament top level width (each slot = 16 columns)
STRIDE = N // TOPW        # 16: original columns of top-level slot p: p + 256k
# max8 group bounds over T4 [0,256). 6 groups of ~43 slots = ~680 original
# columns each; P(a group truly holds >=8 of the top-18) ~ 3.2% of rows,
# which (plus the detector margin) sets the host repair rate.
GB3 = (0, 43, 85, 128, 171, 213, 256)
NG = len(GB3) - 1
UW = NG * 8               # 48 candidate slots per row
MARGIN = 0.5              # hazard detector band (device noise <= ~0.12)


def _build_program(n_tiles=N_TILES):
    nc = bacc.Bacc(
        "TRN2", target_bir_lowering=False, debug=False, enable_asserts=False
    )
    f32 = mybir.dt.float32
    f16 = mybir.dt.float16
    u16 = mybir.dt.uint16
    nq = n_tiles * P
    lhs_a = nc.dram_tensor("lhs_a", (KM, nq), f16, kind="ExternalInput")
    rhs_a = nc.dram_tensor("rhs_a", (KM, N), f16, kind="ExternalInput")
    l_out = nc.dram_tensor("l_out", (nq, UW), u16, kind="ExternalOutput")
    lhs_a_ap, rhs_a_ap = lhs_a.ap(), rhs_a.ap()
    l_ap = l_out.ap()

    with tile.TileContext(nc) as tc:
        with (
            tc.tile_pool(name="const", bufs=1) as cpool,
            tc.tile_pool(name="psum", bufs=2, space="PSUM") as ppool,
            tc.tile_pool(name="csp", bufs=2) as cspool,
            tc.tile_pool(name="t1p", bufs=3) as t1pool,
            tc.tile_pool(name="t2p", bufs=2) as t2pool,
            tc.tile_pool(name="t3p", bufs=2) as t3pool,
            tc.tile_pool(name="outp", bufs=4) as opool,
        ):
            # dependency-free warm-up matmuls that run during the input-DMA
            # prologue (nudges the PE toward its full-rate p-state)
            prime = cpool.tile([KM, 512], f16)
            nc.gpsimd.memset(prime[:, :], 0.0)
            pps = ppool.tile([P, 2048], f32, tag="ps")
            for _ in range(12):
                nc.tensor.matmul(pps[:, :512], prime[:, :128], prime[:, :],
                                 start=True, stop=True)

            # per-512-column-chunk input tiles: the first matmul only waits
            # for its own chunk, not the whole load
            ra_sb = [
                cpool.tile([KM, 512], f16, name=f"ra{j}", tag=f"ra{j}")
                for j in range(8)
            ]
            la_sb = cpool.tile([KM, nq], f16)
            w0 = min(512, nq)
            nc.sync.dma_start(la_sb[:, 0:w0], lhs_a_ap[:, 0:w0])
            nc.sync.dma_start(ra_sb[0][:, :], rhs_a_ap[:, 0:512])
            for j in range(1, 8):
                nc.sync.dma_start(ra_sb[j][:, :], rhs_a_ap[:, j * 512 : (j + 1) * 512])
            for j in range(512, nq, 512):
                w = min(512, nq - j)
                nc.sync.dma_start(la_sb[:, j : j + w], lhs_a_ap[:, j : j + w])

            for t in range(n_tiles):
                qs = slice(t * P, (t + 1) * P)
                t1 = t1pool.tile([P, 2048], f32, tag="t1")
                for h in range(2):
                    ps = ppool.tile([P, 2048], f32, tag="ps")
                    cs = cspool.tile([P, 1024], f32, tag="cs")
                    for j in range(4):
                        cj = h * 4 + j
                        nc.tensor.matmul(
                            ps[:, j * 512 : (j + 1) * 512],
                            la_sb[:, qs], ra_sb[cj][:, :],
                            start=True, stop=True,
                        )
                    # only one TensorTensor operand may live in PSUM: stage
                    # cols 1024:2048 to SBUF on the (otherwise idle) scalar
                    # engine, then L1: 2048 -> 1024 per half on DVE
                    nc.scalar.copy(cs[:, 0:512], ps[:, 1024:1536])
                    nc.scalar.copy(cs[:, 512:1024], ps[:, 1536:2048])
                    nc.vector.tensor_max(
                        t1[:, h * 1024 : (h + 1) * 1024],
                        ps[:, 0:1024], cs[:, :],
                    )

                # L2..L4 on DVE (TensorTensor does not lower on gpsimd)
                t2 = t2pool.tile([P, 1024], f32, tag="t2")
                nc.vector.tensor_max(t2[:, :], t1[:, 0:1024], t1[:, 1024:2048])
                t3 = t3pool.tile([P, 512], f32, tag="t3")
                nc.vector.tensor_max(t3[:, :], t2[:, 0:512], t2[:, 512:1024])
                t4 = t3pool.tile([P, TOPW], f32, tag="t4")
                nc.vector.tensor_max(t4[:, :], t3[:, 0:256], t3[:, 256:512])

                # top-8 per group + first-occurrence local slot index
                u = opool.tile([P, UW], f32, tag="u")
                l = opool.tile([P, UW], u16, tag="l")
                for g in range(NG):
                    nc.vector.max(
                        out=u[:, g * 8 : (g + 1) * 8],
                        in_=t4[:, GB3[g] : GB3[g + 1]],
                    )
                for g in range(NG):
                    nc.vector.max_index(
                        out=l[:, g * 8 : (g + 1) * 8],
                        in_max=u[:, g * 8 : (g + 1) * 8],
                        in_values=t4[:, GB3[g] : GB3[g + 1]],
                    )

                rs = slice(t * P, (t + 1) * P)
                nc.sync.dma_start(l_ap[rs, :], l[:])
    nc.compile()
    return nc


def _prep_core_inputs(X, core):
    """X: (B, N, C) fp32. Returns input map for one core."""
    b, h = divmod(core, N_CORES // B)
    Xb = X[b]
    xsq = np.sum(Xb * Xb, axis=1, dtype=np.float32)
    ch = Xb.T.astype(np.float16)                  # (C, N)
    s1 = (-xsq).astype(np.float16)
    s2 = (-xsq - s1.astype(np.float32)).astype(np.float16)
    rhs_a = np.zeros((KM, N), np.float16)
    rhs_a[:C] = ch
    rhs_a[C] = s1
    rhs_a[C + 1] = s2

    Q = 2.0 * Xb[h * QROWS : (h + 1) * QROWS]     # (QROWS, C)
    qh = Q.T.astype(np.float16)                   # (C, QROWS)
    lhs_a = np.zeros((KM, QROWS), np.float16)
    lhs_a[:C] = qh
    lhs_a[C : C + 2] = 1.0
    return {"lhs_a": lhs_a, "rhs_a": rhs_a}


# slot s -> group g = s//8; T3 position p = GB3[g] + l[s]; columns p + 512k
_GOFF = np.asarray(GB3[:-1], dtype=np.int64)[np.arange(UW) // 8]   # (48,)
_KOFF = (np.arange(STRIDE, dtype=np.int64) * TOPW)                 # (16,)


def _merge_core(L, Xb64, xsq64, q0):
    """L: (R, 48) uint16 local slot indices for queries q0..q0+R-1 of batch b.
    Returns (idx (R,18) int64, flagged-row mask (R,))."""
    R = L.shape[0]
    Ppos = L.astype(np.int64) + _GOFF[None, :]               # (R, 48) in [0,TOPW)
    cols = Ppos[:, :, None] + _KOFF[None, None, :]           # (R, 48, 16)

    # duplicate-slot rule: same T3 position twice within a group
    ps = np.sort(Ppos.reshape(R, NG, 8), axis=2)
    dup = (np.diff(ps, axis=2) == 0).any(axis=(1, 2))

    idx = np.empty((R, K_EFF), np.int64)
    flag = np.empty(R, bool)
    CH = 512
    for c0 in range(0, R, CH):
        c1 = min(c0 + CH, R)
        cc = cols[c0:c1]                                      # (r, 48, 16)
        g = Xb64[cc.reshape(c1 - c0, -1)]                     # (r, 768, 64)
        xq = Xb64[q0 + c0 : q0 + c1]                          # (r, 64)
        vals = 2.0 * np.matmul(g, xq[:, :, None])[:, :, 0]    # (r, 768)
        vals -= xsq64[cc.reshape(c1 - c0, -1)]

        # margin rule: per-slot winner, per-group min of the 8 winners
        w = vals.reshape(c1 - c0, UW, STRIDE).max(axis=2)     # (r, 48)
        gmin = w.reshape(c1 - c0, NG, 8).min(axis=2)          # (r, NG)
        t18 = np.partition(vals, vals.shape[1] - K_EFF, axis=1)[
            :, vals.shape[1] - K_EFF
        ]
        flag[c0:c1] = (gmin >= (t18[:, None] - MARGIN)).any(axis=1)

        # stable top-18 by (value desc, col asc): sort cols ascending first
        fc = cc.reshape(c1 - c0, -1)
        corder = np.argsort(fc, axis=1, kind="stable")
        fc_s = np.take_along_axis(fc, corder, axis=1)
        va_s = np.take_along_axis(vals, corder, axis=1)
        vorder = np.argsort(-va_s, axis=1, kind="stable")[:, :K_EFF]
        idx[c0:c1] = np.take_along_axis(fc_s, vorder, axis=1)
    return idx, (flag | dup)


_NC_CACHE = {}


def kernel(x: np.ndarray) -> np.ndarray:
    x = np.asarray(x)
    assert x.shape == (B, C, N, 1), x.shape
    X = np.ascontiguousarray(np.transpose(x[..., 0], (0, 2, 1)))  # (B, N, C)

    if N_TILES not in _NC_CACHE:
        _NC_CACHE[N_TILES] = _build_program(N_TILES)
    nc = _NC_CACHE[N_TILES]

    in_maps = [_prep_core_inputs(X, c) for c in range(N_CORES)]
    res = run_bass_kernel_spmd(nc, in_maps, core_ids=list(range(N_CORES)))

    X64 = X.astype(np.float64)
    xsq64 = np.einsum("bnc,bnc->bn", X64, X64)

    nn_idx = np.empty((B, N, K_EFF), np.int64)
    bad_rows = [[] for _ in range(B)]
    for core in range(N_CORES):
        b, h = divmod(core, N_CORES // B)
        r = res.results[core]
        idx, bad = _merge_core(r["l_out"], X64[b], xsq64[b], h * QROWS)
        nn_idx[b, h * QROWS : (h + 1) * QROWS] = idx
        if bad.any():
            bad_rows[b].extend((h * QROWS + np.nonzero(bad)[0]).tolist())

    # full exact recompute of flagged rows
    for b in range(B):
        if not bad_rows[b]:
            continue
        rows = np.asarray(sorted(bad_rows[b]))
        S = 2.0 * (X64[b, rows] @ X64[b].T) - xsq64[b][None, :]
        order = np.argsort(-S, axis=1, kind="stable")
        nn_idx[b, rows] = order[:, :K_EFF]

    nn_dil = nn_idx[:, :, ::DILATION]                       # (B, N, 9)
    center = np.broadcast_to(np.arange(N)[None, :, None], nn_dil.shape)
    out = np.stack((nn_dil, center), axis=0).astype(np.int32)
    return out


# revision 8
# speedup vs baseline: 1.5415x; 1.0277x over previous
"""TRN2 Bass kernel for DenseDilatedKnnGraph (B=4, C=64, N=4096, k=9, dilation=2).

Algorithm v2 (tournament-tree candidate selection + exact host rescore)
----------------------------------------------------------------------
reference: xt (B,N,C); dist(i,j) = |xi|^2 - 2<xi,xj> + |xj|^2; nn_idx = top-18
of -dist per row (stable, lowest-index tie-break); output nn_idx[..., ::2] plus
a center-index row -> (2, B, N, 9) int32.

Per-row ordering of -dist equals the ordering of s_ij = 2<xi,xj> - |xj|^2.
The device computes an APPROXIMATE s~ (single fp16 matmul, error ~0.01) that
is only used to SELECT candidate columns; the host rescores candidates in
fp64, so device values never need to be exact.

Device (per core, SPMD over 8 cores; core = (batch, query-half)):
  - s~ via ONE fp16 K=128 matmul into PSUM fp32:
      stationary [qh(64); 1; 1; 0...], moving [ch(64); s1; s2; junk]
    where qh = fp16(2x_q), ch = fp16(x_c), s1+s2 = 2-level fp16 split of
    -|x_c|^2 (junk rows nulled by zero stationary rows). 128-query tiles,
    512-wide PSUM chunks, [128,2048] PSUM buffers x2.
  - Tournament max tree (values preserved exactly through fp32 max):
      L1 (DVE):    T1[j] = max(ps[j], cs[j]) per half, where cs = scalar-engine
                   copy of ps[:, 1024:2048] (the ISA allows only ONE PSUM
                   operand per TensorTensor; tensor_tensor reads 2 ops/cycle:
                   2x cheaper than MAX8 scans)
      L2..L4 (DVE): T2[j] = max(T1[j], T1[j+1024]); T3[j] = max(T2[j],
                   T2[j+512]); T4[j] = max(T3[j], T3[j+256]) -> T4 256 wide,
                   T4[j] covers original columns {j + 256k, k=0..15}.
                   (TensorTensor does not lower on the Pool/GpSimd engine,
                   so the whole tree lives on DVE.)
  - DVE max8 (top-8 values per T4 group) + max_index (first-occurrence local
    slot) on the NARROW T4 only: NG groups over [0,256).
  - DMA out: local slot indices L (128 x NG*8 uint16). Values stay on device.

Host: slot -> T4 position p -> 16 candidate columns {p + 256k}. Rescore all
48*16 = 768 candidates per row exactly (fp64 BLAS), stable top-18 by
(value desc, col asc) == jax.lax.top_k ordering. Soundness: a true top-18
member's T4 slot is either selected (margin ~100 sigma) or its row is flagged
for full exact repair by
  (a) margin rule: some group's 8 slot-winners all within MARGIN=0.5 of the
      row's 18th-best (device noise < 0.12 worst case), or
  (b) duplicate-slot rule: find_index returned the same slot twice (fp32
      value tie collapsed a slot).
Flagged rows (~4-7%) get a full 4096-wide fp64 recompute on host.
"""

import numpy as np

import concourse.bacc as bacc
import concourse.mybir as mybir
import concourse.tile as tile
from concourse.bass_utils import run_bass_kernel_spmd

# Problem constants (hardcoded per harness contract).
B = 4
C = 64
N = 4096
K = 9
DILATION = 2
K_EFF = K * DILATION      # 18
P = 128                   # partitions / queries per tile
KM = 66                   # matmul contraction rows (K>=65 keeps PE full rate;
                          # 66 = 64 q-dims + 2 ones-rows, smaller LDWEIGHTS)
N_CORES = 8
QROWS = (B * N) // N_CORES          # 2048 query rows per core
N_TILES = QROWS // P                # 16 tiles per core

TOPW = 256                # tournament top level width (each slot = 16 columns)
STRIDE = N // TOPW        # 16: original columns of top-level slot p: p + 256k
# max8 group bounds over T4 [0,256). 5 groups of ~51 slots = ~820 original
# columns each; P(a group truly holds >=8 of the top-18) ~ 7.8% of rows,
# which (plus the detector margin) sets the host repair rate.
GB3 = (0, 51, 102, 154, 205, 256)
NG = len(GB3) - 1
UW = NG * 8               # 48 candidate slots per row
MARGIN = 0.5              # hazard detector band (device noise <= ~0.12)


def _build_program(n_tiles=N_TILES):
    nc = bacc.Bacc(
        "TRN2", target_bir_lowering=False, debug=False, enable_asserts=False
    )
    f32 = mybir.dt.float32
    f16 = mybir.dt.float16
    u16 = mybir.dt.uint16
    nq = n_tiles * P
    lhs_a = nc.dram_tensor("lhs_a", (KM, nq), f16, kind="ExternalInput")
    rhs_a = nc.dram_tensor("rhs_a", (KM, N), f16, kind="ExternalInput")
    l_out = nc.dram_tensor("l_out", (nq, UW), u16, kind="ExternalOutput")
    lhs_a_ap, rhs_a_ap = lhs_a.ap(), rhs_a.ap()
    l_ap = l_out.ap()

    with tile.TileContext(nc) as tc:
        with (
            tc.tile_pool(name="const", bufs=1) as cpool,
            tc.tile_pool(name="psum", bufs=2, space="PSUM") as ppool,
            tc.tile_pool(name="csp", bufs=3) as cspool,
            tc.tile_pool(name="t1p", bufs=3) as t1pool,
            tc.tile_pool(name="t2p", bufs=2) as t2pool,
            tc.tile_pool(name="t3p", bufs=2) as t3pool,
            tc.tile_pool(name="outp", bufs=4) as opool,
        ):
            # dependency-free warm-up matmuls that run during the input-DMA
            # prologue (nudges the PE toward its full-rate p-state)
            prime = cpool.tile([KM, 512], f16)
            nc.gpsimd.memset(prime[:, :], 0.0)
            pps = ppool.tile([P, 2048], f32, tag="ps")
            for _ in range(12):
                nc.tensor.matmul(pps[:, :512], prime[:, :128], prime[:, :],
                                 start=True, stop=True)

            # per-512-column-chunk input tiles: the first matmul only waits
            # for its own chunk, not the whole load
            ra_sb = [
                cpool.tile([KM, 512], f16, name=f"ra{j}", tag=f"ra{j}")
                for j in range(8)
            ]
            la_sb = cpool.tile([KM, nq], f16)
            w0 = min(512, nq)
            nc.sync.dma_start(la_sb[:, 0:w0], lhs_a_ap[:, 0:w0])
            nc.sync.dma_start(ra_sb[0][:, :], rhs_a_ap[:, 0:512])
            for j in range(1, 8):
                nc.sync.dma_start(ra_sb[j][:, :], rhs_a_ap[:, j * 512 : (j + 1) * 512])
            for j in range(512, nq, 512):
                w = min(512, nq - j)
                nc.sync.dma_start(la_sb[:, j : j + w], lhs_a_ap[:, j : j + w])

            for t in range(n_tiles):
                qs = slice(t * P, (t + 1) * P)
                t1 = t1pool.tile([P, 2048], f32, tag="t1")
                for h in range(2):
                    ps = ppool.tile([P, 2048], f32, tag="ps")
                    cs = cspool.tile([P, 1024], f32, tag="cs")
                    for j in range(4):
                        cj = h * 4 + j
                        nc.tensor.matmul(
                            ps[:, j * 512 : (j + 1) * 512],
                            la_sb[:, qs], ra_sb[cj][:, :],
                            start=True, stop=True,
                        )
                    # only one TensorTensor operand may live in PSUM: stage
                    # cols 1024:2048 to SBUF on the (otherwise idle) scalar
                    # engine, then L1: 2048 -> 1024 per half on DVE
                    nc.scalar.copy(cs[:, 0:1024], ps[:, 1024:2048])
                    nc.vector.tensor_max(
                        t1[:, h * 1024 : (h + 1) * 1024],
                        ps[:, 0:1024], cs[:, :],
                    )

                # L2..L4 on DVE (TensorTensor does not lower on gpsimd)
                t2 = t2pool.tile([P, 1024], f32, tag="t2")
                nc.vector.tensor_max(t2[:, :], t1[:, 0:1024], t1[:, 1024:2048])
                t3 = t3pool.tile([P, 512], f32, tag="t3")
                nc.vector.tensor_max(t3[:, :], t2[:, 0:512], t2[:, 512:1024])
                t4 = t3pool.tile([P, TOPW], f32, tag="t4")
                nc.vector.tensor_max(t4[:, :], t3[:, 0:256], t3[:, 256:512])

                # top-8 per group + first-occurrence local slot index
                u = opool.tile([P, UW], f32, tag="u")
                l = opool.tile([P, UW], u16, tag="l")
                for g in range(NG):
                    nc.vector.max(
                        out=u[:, g * 8 : (g + 1) * 8],
                        in_=t4[:, GB3[g] : GB3[g + 1]],
                    )
                for g in range(NG):
                    nc.vector.max_index(
                        out=l[:, g * 8 : (g + 1) * 8],
                        in_max=u[:, g * 8 : (g + 1) * 8],
                        in_values=t4[:, GB3[g] : GB3[g + 1]],
                    )

                rs = slice(t * P, (t + 1) * P)
                nc.sync.dma_start(l_ap[rs, :], l[:])
    nc.compile()
    return nc


def _prep_core_inputs(X, core):
    """X: (B, N, C) fp32. Returns input map for one core."""
    b, h = divmod(core, N_CORES // B)
    Xb = X[b]
    xsq = np.sum(Xb * Xb, axis=1, dtype=np.float32)
    ch = Xb.T.astype(np.float16)                  # (C, N)
    s1 = (-xsq).astype(np.float16)
    s2 = (-xsq - s1.astype(np.float32)).astype(np.float16)
    rhs_a = np.zeros((KM, N), np.float16)
    rhs_a[:C] = ch
    rhs_a[C] = s1
    rhs_a[C + 1] = s2

    Q = 2.0 * Xb[h * QROWS : (h + 1) * QROWS]     # (QROWS, C)
    qh = Q.T.astype(np.float16)                   # (C, QROWS)
    lhs_a = np.zeros((KM, QROWS), np.float16)
    lhs_a[:C] = qh
    lhs_a[C : C + 2] = 1.0
    return {"lhs_a": lhs_a, "rhs_a": rhs_a}


# slot s -> group g = s//8; T3 position p = GB3[g] + l[s]; columns p + 512k
_GOFF = np.asarray(GB3[:-1], dtype=np.int64)[np.arange(UW) // 8]   # (48,)
_KOFF = (np.arange(STRIDE, dtype=np.int64) * TOPW)                 # (16,)


def _merge_core(L, Xb64, xsq64, q0):
    """L: (R, 48) uint16 local slot indices for queries q0..q0+R-1 of batch b.
    Returns (idx (R,18) int64, flagged-row mask (R,))."""
    R = L.shape[0]
    Ppos = L.astype(np.int64) + _GOFF[None, :]               # (R, 48) in [0,TOPW)
    cols = Ppos[:, :, None] + _KOFF[None, None, :]           # (R, 48, 16)

    # duplicate-slot rule: same T3 position twice within a group
    ps = np.sort(Ppos.reshape(R, NG, 8), axis=2)
    dup = (np.diff(ps, axis=2) == 0).any(axis=(1, 2))

    idx = np.empty((R, K_EFF), np.int64)
    flag = np.empty(R, bool)
    CH = 512
    for c0 in range(0, R, CH):
        c1 = min(c0 + CH, R)
        cc = cols[c0:c1]                                      # (r, 48, 16)
        g = Xb64[cc.reshape(c1 - c0, -1)]                     # (r, 768, 64)
        xq = Xb64[q0 + c0 : q0 + c1]                          # (r, 64)
        vals = 2.0 * np.matmul(g, xq[:, :, None])[:, :, 0]    # (r, 768)
        vals -= xsq64[cc.reshape(c1 - c0, -1)]

        # margin rule: per-slot winner, per-group min of the 8 winners
        w = vals.reshape(c1 - c0, UW, STRIDE).max(axis=2)     # (r, 48)
        gmin = w.reshape(c1 - c0, NG, 8).min(axis=2)          # (r, NG)
        t18 = np.partition(vals, vals.shape[1] - K_EFF, axis=1)[
            :, vals.shape[1] - K_EFF
        ]
        flag[c0:c1] = (gmin >= (t18[:, None] - MARGIN)).any(axis=1)

        # stable top-18 by (value desc, col asc): sort cols ascending first
        fc = cc.reshape(c1 - c0, -1)
        corder = np.argsort(fc, axis=1, kind="stable")
        fc_s = np.take_along_axis(fc, corder, axis=1)
        va_s = np.take_along_axis(vals, corder, axis=1)
        vorder = np.argsort(-va_s, axis=1, kind="stable")[:, :K_EFF]
        idx[c0:c1] = np.take_along_axis(fc_s, vorder, axis=1)
    return idx, (flag | dup)


_NC_CACHE = {}


def kernel(x: np.ndarray) -> np.ndarray:
    x = np.asarray(x)
    assert x.shape == (B, C, N, 1), x.shape
    X = np.ascontiguousarray(np.transpose(x[..., 0], (0, 2, 1)))  # (B, N, C)

    if N_TILES not in _NC_CACHE:
        _NC_CACHE[N_TILES] = _build_program(N_TILES)
    nc = _NC_CACHE[N_TILES]

    in_maps = [_prep_core_inputs(X, c) for c in range(N_CORES)]
    res = run_bass_kernel_spmd(nc, in_maps, core_ids=list(range(N_CORES)))

    X64 = X.astype(np.float64)
    xsq64 = np.einsum("bnc,bnc->bn", X64, X64)

    nn_idx = np.empty((B, N, K_EFF), np.int64)
    bad_rows = [[] for _ in range(B)]
    for core in range(N_CORES):
        b, h = divmod(core, N_CORES // B)
        r = res.results[core]
        idx, bad = _merge_core(r["l_out"], X64[b], xsq64[b], h * QROWS)
        nn_idx[b, h * QROWS : (h + 1) * QROWS] = idx
        if bad.any():
            bad_rows[b].extend((h * QROWS + np.nonzero(bad)[0]).tolist())

    # full exact recompute of flagged rows
    for b in range(B):
        if not bad_rows[b]:
            continue
        rows = np.asarray(sorted(bad_rows[b]))
        S = 2.0 * (X64[b, rows] @ X64[b].T) - xsq64[b][None, :]
        order = np.argsort(-S, axis=1, kind="stable")
        nn_idx[b, rows] = order[:, :K_EFF]

    nn_dil = nn_idx[:, :, ::DILATION]                       # (B, N, 9)
    center = np.broadcast_to(np.arange(N)[None, :, None], nn_dil.shape)
    out = np.stack((nn_dil, center), axis=0).astype(np.int32)
    return out


# revision 14
# speedup vs baseline: 1.7484x; 1.1343x over previous
"""TRN2 Bass kernel for DenseDilatedKnnGraph (B=4, C=64, N=4096, k=9, dilation=2).

Algorithm v2 (tournament-tree candidate selection + exact host rescore)
----------------------------------------------------------------------
reference: xt (B,N,C); dist(i,j) = |xi|^2 - 2<xi,xj> + |xj|^2; nn_idx = top-18
of -dist per row (stable, lowest-index tie-break); output nn_idx[..., ::2] plus
a center-index row -> (2, B, N, 9) int32.

Per-row ordering of -dist equals the ordering of s_ij = 2<xi,xj> - |xj|^2.
The device computes an APPROXIMATE s~ (single fp16 matmul, error ~0.01) that
is only used to SELECT candidate columns; the host rescores candidates in
fp64, so device values never need to be exact.

Device (per core, SPMD over 8 cores; core = (batch, query-half)):
  - v~ = ALPHA*s~ + beta_q via ONE fp16 K=128 matmul into PSUM fp32:
      stationary [ALPHA*2x_q (64); 1; 1; beta_q/2; beta_q/2; 0...],
      moving    [x_c (64); -ALPHA*|x_c|^2/2 (x2); 1; 1; junk]
    beta_q = VCENTER + ALPHA*(dist_est_q - |x_q|^2) places each row's
    nearest-neighbor region near VCENTER..VCENTER+10*ALPHA on a uint16 grid
    (dist_est_q = sampled-min distance estimate, host-computed). The self
    match (dist 0) saturates to 65535, far columns clamp to 0 -- both are
    handled by the host detectors. 128-query tiles, 512-wide PSUM chunks,
    [128,2048] PSUM buffers x2.
  - Tournament max tree in uint16 (monotone fp32->uint16 cast commutes with
    max, so the tree equals uint16-cast maxima exactly; 2-byte dtype runs the
    upper tree at the DVE 2x rate):
      L1 (DVE):    T1[j] = max(ps[j], cs[j]) per half, where cs = scalar-engine
                   copy of ps[:, 1024:2048] (the ISA allows only ONE PSUM
                   operand per TensorTensor; tensor_tensor reads 2 ops/cycle:
                   2x cheaper than MAX8 scans)
      L2..L4 (DVE): T2[j] = max(T1[j], T1[j+1024]); T3[j] = max(T2[j],
                   T2[j+512]); T4[j] = max(T3[j], T3[j+256]) -> T4 256 wide,
                   T4[j] covers original columns {j + 256k, k=0..15}.
                   (TensorTensor does not lower on the Pool/GpSimd engine,
                   so the whole tree lives on DVE.)
  - DVE max8 (top-8 values per T4 group) + max_index (first-occurrence local
    slot) on the NARROW T4 only: NG groups over [0,256).
  - DMA out: local slot indices L (128 x NG*8 uint16). Values stay on device.

Host: slot -> T4 position p -> 16 candidate columns {p + 256k}. Rescore all
48*16 = 768 candidates per row exactly (fp64 BLAS), stable top-18 by
(value desc, col asc) == jax.lax.top_k ordering. Soundness: a true top-18
member's T4 slot is either selected (margin ~100 sigma) or its row is flagged
for full exact repair by
  (a) margin rule: some group's 8 slot-winners all within MARGIN=0.5 of the
      row's 18th-best (device noise < 0.12 worst case), or
  (b) duplicate-slot rule: find_index returned the same slot twice (fp32
      value tie collapsed a slot).
Flagged rows (~4-7%) get a full 4096-wide fp64 recompute on host.
"""

import numpy as np

import concourse.bacc as bacc
import concourse.mybir as mybir
import concourse.tile as tile
from concourse.bass_utils import run_bass_kernel_spmd

# Problem constants (hardcoded per harness contract).
B = 4
C = 64
N = 4096
K = 9
DILATION = 2
K_EFF = K * DILATION      # 18
P = 128                   # partitions / queries per tile
KM = 128                  # matmul contraction rows (K=66 measured ~10% slower
                          # per matmul than K=128, so keep 128)
N_CORES = 8
QROWS = (B * N) // N_CORES          # 2048 query rows per core
N_TILES = QROWS // P                # 16 tiles per core

TOPW = 256                # tournament top level width (each slot = 16 columns)
STRIDE = N // TOPW        # 16: original columns of top-level slot p: p + 256k
# max8 group bounds over T4 [0,256). 5 groups of ~51 slots = ~820 original
# columns each; P(a group truly holds >=8 of the top-18) ~ 7.8% of rows,
# which (plus the detector margin) sets the host repair rate.
GB3 = (0, 51, 102, 154, 205, 256)
NG = len(GB3) - 1
UW = NG * 8               # 48 candidate slots per row
MARGIN = 0.5              # hazard detector band (device noise <= ~0.12)
ALPHA0 = 1000.0           # fixed scale baked into the shared -|x_c|^2 rows
VCENTER = 24000.0         # grid value at dist == dist_est
SELF_V = 65100.0          # grid value the self match (dist 0) is pinned to;
                          # per-row alpha_q = (SELF_V-VCENTER)/dist_est makes
                          # v < SELF_V for every dist > 0, so nothing can
                          # wrap past 65535 (hw cast wraps, not saturates)


def _build_program(n_tiles=N_TILES):
    nc = bacc.Bacc(
        "TRN2", target_bir_lowering=False, debug=False, enable_asserts=False
    )
    f32 = mybir.dt.float32
    f16 = mybir.dt.float16
    u16 = mybir.dt.uint16
    nq = n_tiles * P
    lhs_a = nc.dram_tensor("lhs_a", (KM, nq), f16, kind="ExternalInput")
    rhs_a = nc.dram_tensor("rhs_a", (KM, N), f16, kind="ExternalInput")
    l_out = nc.dram_tensor("l_out", (nq, UW), u16, kind="ExternalOutput")
    lhs_a_ap, rhs_a_ap = lhs_a.ap(), rhs_a.ap()
    l_ap = l_out.ap()

    with tile.TileContext(nc) as tc:
        with (
            tc.tile_pool(name="const", bufs=1) as cpool,
            tc.tile_pool(name="psum", bufs=2, space="PSUM") as ppool,
            tc.tile_pool(name="csp", bufs=3) as cspool,
            tc.tile_pool(name="t1p", bufs=3) as t1pool,
            tc.tile_pool(name="t2p", bufs=2) as t2pool,
            tc.tile_pool(name="t3p", bufs=2) as t3pool,
            tc.tile_pool(name="outp", bufs=4) as opool,
        ):
            # dependency-free warm-up matmuls that run during the input-DMA
            # prologue (nudges the PE toward its full-rate p-state)
            prime = cpool.tile([KM, 512], f16)
            nc.gpsimd.memset(prime[:, :], 0.0)
            pps = ppool.tile([P, 2048], f32, tag="ps")
            for _ in range(5):
                nc.tensor.matmul(pps[:, :512], prime[:, :128], prime[:, :],
                                 start=True, stop=True)

            # per-512-column-chunk input tiles: the first matmul only waits
            # for its own chunk, not the whole load
            ra_sb = [
                cpool.tile([KM, 512], f16, name=f"ra{j}", tag=f"ra{j}")
                for j in range(8)
            ]
            la_sb = cpool.tile([KM, nq], f16)
            w0 = min(512, nq)
            nc.sync.dma_start(la_sb[:, 0:w0], lhs_a_ap[:, 0:w0])
            nc.sync.dma_start(ra_sb[0][:, :], rhs_a_ap[:, 0:512])
            for j in range(1, 8):
                nc.sync.dma_start(ra_sb[j][:, :], rhs_a_ap[:, j * 512 : (j + 1) * 512])
            for j in range(512, nq, 512):
                w = min(512, nq - j)
                nc.sync.dma_start(la_sb[:, j : j + w], lhs_a_ap[:, j : j + w])

            for t in range(n_tiles):
                qs = slice(t * P, (t + 1) * P)
                t1 = t1pool.tile([P, 2048], u16, tag="t1")
                for h in range(2):
                    ps = ppool.tile([P, 2048], f32, tag="ps")
                    cs = cspool.tile([P, 1024], f32, tag="cs")
                    for j in range(4):
                        cj = h * 4 + j
                        nc.tensor.matmul(
                            ps[:, j * 512 : (j + 1) * 512],
                            la_sb[:, qs], ra_sb[cj][:, :],
                            start=True, stop=True,
                        )
                    # only one TensorTensor operand may live in PSUM: stage
                    # cols 1024:2048 to SBUF on the (otherwise idle) scalar
                    # engine, then L1: 2048 -> 1024 per half on DVE
                    nc.scalar.copy(cs[:, 0:1024], ps[:, 1024:2048])
                    nc.vector.tensor_max(
                        t1[:, h * 1024 : (h + 1) * 1024],
                        ps[:, 0:1024], cs[:, :],
                    )

                # L2..L4 on DVE (TensorTensor does not lower on gpsimd)
                t2 = t2pool.tile([P, 1024], u16, tag="t2")
                nc.vector.tensor_max(t2[:, :], t1[:, 0:1024], t1[:, 1024:2048])
                t3 = t3pool.tile([P, 512], u16, tag="t3")
                nc.vector.tensor_max(t3[:, :], t2[:, 0:512], t2[:, 512:1024])
                t4 = t3pool.tile([P, TOPW], u16, tag="t4")
                nc.vector.tensor_max(t4[:, :], t3[:, 0:256], t3[:, 256:512])

                # top-8 per group + first-occurrence local slot index
                u = opool.tile([P, UW], u16, tag="u")
                l = opool.tile([P, UW], u16, tag="l")
                for g in range(NG):
                    nc.vector.max(
                        out=u[:, g * 8 : (g + 1) * 8],
                        in_=t4[:, GB3[g] : GB3[g + 1]],
                    )
                for g in range(NG):
                    nc.vector.max_index(
                        out=l[:, g * 8 : (g + 1) * 8],
                        in_max=u[:, g * 8 : (g + 1) * 8],
                        in_values=t4[:, GB3[g] : GB3[g + 1]],
                    )

                rs = slice(t * P, (t + 1) * P)
                nc.sync.dma_start(l_ap[rs, :], l[:])
    nc.compile()
    return nc


_SAMPLE_COLS = np.arange(11, N, 21)[:192]         # 192 fixed probe columns


def _prep_core_inputs(X, core):
    """X: (B, N, C) fp32. Returns input map for one core."""
    b, h = divmod(core, N_CORES // B)
    Xb = X[b]
    xsq = np.sum(Xb * Xb, axis=1, dtype=np.float32)
    ch = Xb.T.astype(np.float16)                  # (C, N)
    half_s = (-(ALPHA0 * 0.5) * xsq).astype(np.float16)
    rhs_a = np.zeros((KM, N), np.float16)
    rhs_a[:C] = ch
    rhs_a[C] = half_s
    rhs_a[C + 1] = half_s
    rhs_a[C + 2 : C + 4] = 1.0

    Q = Xb[h * QROWS : (h + 1) * QROWS]           # (QROWS, C)
    qsq = xsq[h * QROWS : (h + 1) * QROWS]
    # sampled nearest-distance estimate per query (approximate is fine: only
    # resolution depends on it, never clamp-soundness). The diagonal must be
    # masked: a probe column equal to the query itself gives dist 0 and a
    # garbage estimate.
    dprobe = (
        qsq[:, None]
        + xsq[_SAMPLE_COLS][None, :]
        - 2.0 * (Q @ Xb[_SAMPLE_COLS].T)
    )
    qglob = h * QROWS + np.arange(QROWS)
    dprobe[qglob[:, None] == _SAMPLE_COLS[None, :]] = np.inf
    dsamp = dprobe.min(axis=1)
    dist_est = np.maximum(dsamp, (SELF_V - VCENTER) / 1024.0 + 0.1)
    alpha_q = (SELF_V - VCENTER) / dist_est       # per-row scale, <= 1024
    beta = SELF_V - alpha_q * qsq                 # v_self == SELF_V exactly
    qh = ((2.0 * alpha_q)[None, :] * Q.T).astype(np.float16)
    lhs_a = np.zeros((KM, QROWS), np.float16)
    lhs_a[:C] = qh
    lhs_a[C : C + 2] = (alpha_q / ALPHA0).astype(np.float16)
    lhs_a[C + 2 : C + 4] = (0.5 * beta).astype(np.float16)
    return {"lhs_a": lhs_a, "rhs_a": rhs_a}


# slot s -> group g = s//8; T3 position p = GB3[g] + l[s]; columns p + 512k
_GOFF = np.asarray(GB3[:-1], dtype=np.int64)[np.arange(UW) // 8]   # (48,)
_KOFF = (np.arange(STRIDE, dtype=np.int64) * TOPW)                 # (16,)


def _merge_core(L, Xb64, xsq64, q0):
    """L: (R, 48) uint16 local slot indices for queries q0..q0+R-1 of batch b.
    Returns (idx (R,18) int64, flagged-row mask (R,))."""
    R = L.shape[0]
    Ppos = L.astype(np.int64) + _GOFF[None, :]               # (R, 48) in [0,TOPW)
    cols = Ppos[:, :, None] + _KOFF[None, None, :]           # (R, 48, 16)

    # duplicate-slot rule: same T3 position twice within a group
    ps = np.sort(Ppos.reshape(R, NG, 8), axis=2)
    dup = (np.diff(ps, axis=2) == 0).any(axis=(1, 2))

    idx = np.empty((R, K_EFF), np.int64)
    flag = np.empty(R, bool)
    CH = 512
    NC_ = UW * STRIDE
    for c0 in range(0, R, CH):
        c1 = min(c0 + CH, R)
        cc = cols[c0:c1].reshape(c1 - c0, NC_)                # (r, 768)
        # the self column always belongs to the true top-18 (dist 0) but its
        # on-device value wraps mod 2^16 -- inject it unconditionally
        selfc = np.arange(q0 + c0, q0 + c1, dtype=np.int64)[:, None]
        cc = np.concatenate([cc, selfc], axis=1)              # (r, 769)
        g = Xb64[cc]                                          # (r, 769, 64)
        xq = Xb64[q0 + c0 : q0 + c1]                          # (r, 64)
        vals = 2.0 * np.matmul(g, xq[:, :, None])[:, :, 0]    # (r, 769)
        vals -= xsq64[cc]

        # margin rule: per-slot winner, per-group min of the 8 winners
        # (device slots only -- exclude the injected self column)
        w = vals[:, :NC_].reshape(c1 - c0, UW, STRIDE).max(axis=2)
        gmin = w.reshape(c1 - c0, NG, 8).min(axis=2)          # (r, NG)
        t18 = np.partition(vals, vals.shape[1] - K_EFF, axis=1)[
            :, vals.shape[1] - K_EFF
        ]
        flag[c0:c1] = (gmin >= (t18[:, None] - MARGIN)).any(axis=1)

        # the self col may duplicate a device candidate: mask the device copy
        dupself = cc[:, :NC_] == selfc
        vals[:, :NC_][dupself] = -np.inf

        # stable top-18 by (value desc, col asc): sort cols ascending first
        corder = np.argsort(cc, axis=1, kind="stable")
        fc_s = np.take_along_axis(cc, corder, axis=1)
        va_s = np.take_along_axis(vals, corder, axis=1)
        vorder = np.argsort(-va_s, axis=1, kind="stable")[:, :K_EFF]
        idx[c0:c1] = np.take_along_axis(fc_s, vorder, axis=1)
    return idx, (flag | dup)


_NC_CACHE = {}


def kernel(x: np.ndarray) -> np.ndarray:
    x = np.asarray(x)
    assert x.shape == (B, C, N, 1), x.shape
    X = np.ascontiguousarray(np.transpose(x[..., 0], (0, 2, 1)))  # (B, N, C)

    if N_TILES not in _NC_CACHE:
        _NC_CACHE[N_TILES] = _build_program(N_TILES)
    nc = _NC_CACHE[N_TILES]

    in_maps = [_prep_core_inputs(X, c) for c in range(N_CORES)]
    res = run_bass_kernel_spmd(nc, in_maps, core_ids=list(range(N_CORES)))

    X64 = X.astype(np.float64)
    xsq64 = np.einsum("bnc,bnc->bn", X64, X64)

    nn_idx = np.empty((B, N, K_EFF), np.int64)
    bad_rows = [[] for _ in range(B)]
    for core in range(N_CORES):
        b, h = divmod(core, N_CORES // B)
        r = res.results[core]
        idx, bad = _merge_core(r["l_out"], X64[b], xsq64[b], h * QROWS)
        nn_idx[b, h * QROWS : (h + 1) * QROWS] = idx
        if bad.any():
            bad_rows[b].extend((h * QROWS + np.nonzero(bad)[0]).tolist())

    # full exact recompute of flagged rows
    for b in range(B):
        if not bad_rows[b]:
            continue
        rows = np.asarray(sorted(bad_rows[b]))
        S = 2.0 * (X64[b, rows] @ X64[b].T) - xsq64[b][None, :]
        order = np.argsort(-S, axis=1, kind="stable")
        nn_idx[b, rows] = order[:, :K_EFF]

    nn_dil = nn_idx[:, :, ::DILATION]                       # (B, N, 9)
    center = np.broadcast_to(np.arange(N)[None, :, None], nn_dil.shape)
    out = np.stack((nn_dil, center), axis=0).astype(np.int32)
    return out
